# revision 17
# baseline (speedup 1.0000x reference)
"""Trainium2 Bass kernel for masked-GRU + LayerNorm (nn_GRUModule_32985348833742).

Layout strategy (per core, batch-sharded 8 ways, BS=32 batch each):
  - "Folded" on-chip layout for all H-sized per-step tensors:
      tile[p, 32*k + c] = value[128*k + p, c]   (p: partition, k: 0..3, c: batch)
    so a (512, 32) matrix lives in a single (128, 128) tile.  The 4 moving
    operands for the recurrent matmul are just free-dim slices of that tile.
  - Phase 1 (PE, batched): gi = x @ W_ih^T + b  for 16-step chunks, kept in SBUF.
  - Phase 2 (sequential): per step, gh^T = W_hh @ g via 48 (LDW+MM) pairs with
    bf16 stationary weights (FWL), gates on DVE/ACT in folded layout.
  - Phase 3 (lagged 4-8 steps): PE-transpose h_t, LayerNorm stats via a 0/1
    "S" matmul for the cross-partition reduction, 2 fused STT ops for the
    normalization, DMA out.
Phases are interleaved per-step so phase 1/3 fill PE/DVE bubbles left by the
latency-bound recurrent chain.
"""

import sys
from contextlib import ExitStack

sys.path.insert(0, "/opt/trn_rl_repo")

import numpy as np
import ml_dtypes

import concourse.bass as bass
import concourse.mybir as mybir
import concourse.tile as tile
from concourse import bacc
from concourse.bass_utils import run_bass_kernel_spmd

F32 = mybir.dt.float32
BF16 = mybir.dt.bfloat16
AF = mybir.ActivationFunctionType
ALU = mybir.AluOpType

T, B, I, H = 256, 256, 512, 512
NCORES = 8
BS = B // NCORES          # 32
KT = I // 128             # 4 k-tiles (contraction)
MT = 3 * H // 128         # 12 m-tiles (gate rows)
TC = 16                   # timesteps per gi chunk
NCH = T // TC             # 16 chunks
LN_EPS = 1e-5
P3_LAG = 8                # steps of lag before LN group is emitted
BF = ml_dtypes.bfloat16


def build_program(T=T, TC=TC, gi_preload=True, reps=1, **_unused):
    NCH = T // TC
    nc = bacc.Bacc(None, target_bir_lowering=False, debug=False)

    x_d = nc.dram_tensor("x_fold", [T, 128, 128], BF16, kind="ExternalInput")
    m_d = nc.dram_tensor("m_fold", [T, 128, 128], F32, kind="ExternalInput")
    h0_d = nc.dram_tensor("h0_fold", [128, 128], F32, kind="ExternalInput")
    wih_d = nc.dram_tensor("wihT", [I, 3 * H], BF16, kind="ExternalInput")
    whh_d = nc.dram_tensor("whhT", [H, 3 * H], BF16, kind="ExternalInput")
    blhs_d = nc.dram_tensor("bias_lhs", [1, MT * 128], BF16, kind="ExternalInput")
    bn_d = nc.dram_tensor("bn_fold", [128, 128], F32, kind="ExternalInput")
    smat_d = nc.dram_tensor("smat", [128, 128], F32, kind="ExternalInput")
    wt_d = nc.dram_tensor("wt_fold", [128, 128], F32, kind="ExternalInput")
    bt_d = nc.dram_tensor("bt_fold", [128, 128], F32, kind="ExternalInput")
    id_d = nc.dram_tensor("ident", [128, 128], F32, kind="ExternalInput")
    y_d = nc.dram_tensor("y_out", [T, BS, H], F32, kind="ExternalOutput")
    hT_d = nc.dram_tensor("hT_out", [128, 128], F32, kind="ExternalOutput")

    with tile.TileContext(nc) as tc, ExitStack() as es:
        cpool = es.enter_context(tc.tile_pool(name="consts", bufs=1))
        gi_pool = es.enter_context(tc.tile_pool(name="gi", bufs=2))
        x_pool = es.enter_context(tc.tile_pool(name="xc", bufs=3))
        m_pool = es.enter_context(tc.tile_pool(name="mc", bufs=2))
        wk_pool = es.enter_context(tc.tile_pool(name="work", bufs=3))
        h_pool = es.enter_context(tc.tile_pool(name="hst", bufs=26))
        y_pool = es.enter_context(tc.tile_pool(name="yst", bufs=2))
        ps1_pool = es.enter_context(tc.tile_pool(name="ps1", bufs=2, space="PSUM"))
        ps2_pool = es.enter_context(tc.tile_pool(name="ps2", bufs=2, space="PSUM"))
        ps3_pool = es.enter_context(tc.tile_pool(name="ps3", bufs=1, space="PSUM"))
        ps4_pool = es.enter_context(tc.tile_pool(name="ps4", bufs=2, space="PSUM"))

        # ---- constants ----
        wih_sb = []
        whh_sb = []
        for k in range(KT):
            wt_ih = cpool.tile([128, 3 * H], BF16, tag=f"wih{k}")
            nc.gpsimd.dma_start(wt_ih[:], wih_d[128 * k:128 * (k + 1), :])
            wih_sb.append(wt_ih)
            wt_hh = cpool.tile([128, 3 * H], BF16, tag=f"whh{k}")
            nc.gpsimd.dma_start(wt_hh[:], whh_d[128 * k:128 * (k + 1), :])
            whh_sb.append(wt_hh)
        bias_lhs_sb = cpool.tile([1, MT * 128], BF16, tag="biaslhs")
        nc.gpsimd.dma_start(bias_lhs_sb[:], blhs_d[:])
        ones_sb = cpool.tile([1, TC * 32], BF16, tag="ones")
        nc.gpsimd.memset(ones_sb[:], 1.0)
        bn_sb = cpool.tile([128, 128], F32, tag="bn")
        nc.gpsimd.dma_start(bn_sb[:], bn_d[:])
        smat_sb = cpool.tile([128, 128], F32, tag="smat")
        nc.gpsimd.dma_start(smat_sb[:], smat_d[:])
        wt_sb = cpool.tile([128, 128], F32, tag="wt")
        nc.gpsimd.dma_start(wt_sb[:], wt_d[:])
        bt_sb = cpool.tile([128, 128], F32, tag="bt")
        nc.gpsimd.dma_start(bt_sb[:], bt_d[:])
        id_sb = cpool.tile([128, 128], F32, tag="ident")
        nc.gpsimd.dma_start(id_sb[:], id_d[:])

        # ---- initial state (re-done per rep; reps>1 only for benchmarking) ----
        h_cur = None
        gi_tiles = {}
        x_tiles = {}
        m_tiles = {}

        def load_chunk_inputs(ci):
            if ci >= NCH:
                return
            xt = x_pool.tile([128, TC, 128], BF16, name=f"xchunk{ci}", tag="xchunk")
            nc.gpsimd.dma_start(
                xt[:], x_d[TC * ci:TC * (ci + 1)].rearrange("t p f -> p t f")
            )
            x_tiles[ci] = xt
            mt = m_pool.tile([128, TC, 128], F32, name=f"mchunk{ci}", tag="mchunk")
            nc.gpsimd.dma_start(
                mt[:], m_d[TC * ci:TC * (ci + 1)].rearrange("t p f -> p t f")
            )
            m_tiles[ci] = mt

        def phase1_block(ci, m):
            """gi[:, :, 32m:32m+32] for chunk ci (one m-tile, all TC steps)."""
            if ci >= NCH:
                return
            if m == 0:
                gi_tiles[ci] = gi_pool.tile([128, TC, 384], F32, name=f"gi{ci}", tag="gi")
            git = gi_tiles[ci]
            xt = x_tiles[ci]
            ps = ps1_pool.tile([128, TC, 32], F32, tag="ps1")
            for k in range(KT):
                nc.tensor.matmul(
                    ps[:],
                    wih_sb[k][:, 128 * m:128 * (m + 1)],
                    xt[:, :, 32 * k:32 * (k + 1)],
                    start=(k == 0),
                    stop=False,
                )
            nc.tensor.matmul(
                ps[:], bias_lhs_sb[:, 128 * m:128 * (m + 1)],
                ones_sb[:, 0:TC * 32].rearrange("o (t c) -> o t c", c=32),
                start=False, stop=True,
            )
            nc.scalar.copy(git[:, 0:TC // 2, 32 * m:32 * (m + 1)], ps[:, 0:TC // 2, :])
            nc.scalar.copy(git[:, TC // 2:TC, 32 * m:32 * (m + 1)], ps[:, TC // 2:TC, :])

        def mm_part(ps2, gbf, ms):
            for m in ms:
                for k in range(KT):
                    nc.tensor.matmul(
                        ps2[:, 32 * m:32 * (m + 1)],
                        whh_sb[k][:, 128 * m:128 * (m + 1)],
                        gbf[:, 32 * k:32 * (k + 1)],
                        start=(k == 0 and not gi_preload),
                        stop=(k == KT - 1),
                        skip_group_check=gi_preload,
                    )

        def phase2_step(t):
            nonlocal h_cur
            ci, tl = divmod(t, TC)
            git = gi_tiles[ci]
            mt = m_tiles[ci]
            # masked state, bf16 for the matmul + fp32 for the blend
            gbf = wk_pool.tile([128, 128], BF16, tag="gbf")
            nc.vector.tensor_tensor(gbf[:], h_cur[:], mt[:, tl, :], ALU.mult)
            g32 = wk_pool.tile([128, 128], F32, tag="g32")
            nc.gpsimd.tensor_tensor(g32[:], h_cur[:], mt[:, tl, :], ALU.mult)
            # gh^T = W_hh @ g (folded); m-order r(0-3), n(8-11), z(4-7) so the
            # r->n->tanh chain starts as early as possible
            ps2 = ps2_pool.tile([128, 384], F32, tag="ps2")
            if gi_preload:
                # preload gi_rz and b_hh_n into PSUM via identity matmuls so the
                # W_hh matmuls accumulate on top (PE writes set has_written)
                nc.tensor.matmul(ps2[:, 0:256], id_sb[:], git[:, tl, 0:256],
                                 start=True, stop=False, skip_group_check=True)
                nc.tensor.matmul(ps2[:, 256:384], id_sb[:], bn_sb[:],
                                 start=False, stop=False, skip_group_check=True)
            mm_part(ps2, gbf, (0, 1, 2, 3))
            # r = sigmoid(gi_r + gh_r)
            r_t = wk_pool.tile([128, 128], F32, tag="r")
            if gi_preload:
                nc.scalar.activation(r_t[:], ps2[:, 0:128], AF.Sigmoid)
            else:
                rs = wk_pool.tile([128, 128], F32, tag="rs")
                nc.vector.tensor_tensor(rs[:], ps2[:, 0:128], git[:, tl, 0:128], ALU.add)
                r_t = wk_pool.tile([128, 128], F32, tag="r")
                nc.scalar.activation(r_t[:], rs[:], AF.Sigmoid)
            mm_part(ps2, gbf, (8, 9, 10, 11))
            # n = tanh(gi_n + r * (gh_n + b_hh_n))
            rn = wk_pool.tile([128, 128], F32, tag="rn")
            if gi_preload:
                nc.vector.tensor_tensor(rn[:], r_t[:], ps2[:, 256:384], ALU.mult)
            else:
                hb = wk_pool.tile([128, 128], F32, tag="hb")
                nc.vector.tensor_tensor(hb[:], ps2[:, 256:384], bn_sb[:], ALU.add)
                nc.vector.tensor_tensor(rn[:], r_t[:], hb[:], ALU.mult)
            npre = wk_pool.tile([128, 128], F32, tag="npre")
            nc.vector.tensor_tensor(npre[:], rn[:], git[:, tl, 256:384], ALU.add)
            nn_t = wk_pool.tile([128, 128], F32, tag="nn")
            nc.scalar.activation(nn_t[:], npre[:], AF.Tanh)
            mm_part(ps2, gbf, (4, 5, 6, 7))
            # omz = 1 - z = sigmoid(-(gi_z + gh_z));  h' = g - omz*(g - n)
            omz = wk_pool.tile([128, 128], F32, tag="omz")
            if gi_preload:
                nc.scalar.activation(omz[:], ps2[:, 128:256], AF.Sigmoid, scale=-1.0)
            else:
                zs = wk_pool.tile([128, 128], F32, tag="zs")
                nc.vector.tensor_tensor(zs[:], ps2[:, 128:256], git[:, tl, 128:256], ALU.add)
                nc.scalar.activation(omz[:], zs[:], AF.Sigmoid, scale=-1.0)
            d_t = wk_pool.tile([128, 128], F32, tag="d")
            nc.vector.tensor_tensor(d_t[:], g32[:], nn_t[:], ALU.subtract)
            e_t = wk_pool.tile([128, 128], F32, tag="e")
            nc.vector.tensor_tensor(e_t[:], omz[:], d_t[:], ALU.mult)
            h_new = h_pool.tile([128, 128], F32, tag="h")
            nc.vector.tensor_tensor(h_new[:], g32[:], e_t[:], ALU.subtract)
            h_hist[t] = h_new
            h_cur = h_new

        G = 8  # LN group size

        def phase3_group(gs):
            """LayerNorm + output for steps gs..gs+G-1."""
            hs = [h_hist.pop(gs + j) for j in range(G)]
            ps3 = ps3_pool.tile([128, G, 128], F32, tag="ps3")
            for j in range(G):
                nc.tensor.transpose(ps3[:, j, :], hs[j][:], id_sb[:])
            tr_sb = y_pool.tile([128, G, 128], F32, tag="trsb")
            nc.scalar.copy(tr_sb[:, 0:G // 2, :], ps3[:, 0:G // 2, :])
            nc.scalar.copy(tr_sb[:, G // 2:G, :], ps3[:, G // 2:G, :])
            sq = y_pool.tile([128, G, 128], F32, tag="sq")
            nc.scalar.activation(sq[:, 0:G // 2, :], ps3[:, 0:G // 2, :], AF.Square)
            nc.scalar.activation(sq[:, G // 2:G, :], ps3[:, G // 2:G, :], AF.Square)
            st_in = y_pool.tile([128, 2 * G], F32, tag="stin")
            nc.vector.tensor_reduce(st_in[:, 0:G], ps3[:], mybir.AxisListType.X, ALU.add)
            nc.vector.tensor_reduce(st_in[:, G:2 * G], sq[:], mybir.AxisListType.X, ALU.add)
            ps4 = ps4_pool.tile([128, 2 * G], F32, tag="ps4")
            nc.tensor.matmul(ps4[:], smat_sb[:], st_in[:], start=True, stop=True)
            mu = y_pool.tile([128, G], F32, tag="mu")
            nc.vector.tensor_scalar_mul(mu[:], ps4[:, 0:G], 1.0 / H)
            ex2 = y_pool.tile([128, G], F32, tag="ex2")
            nc.vector.tensor_scalar_mul(ex2[:], ps4[:, G:2 * G], 1.0 / H)
            mu2 = y_pool.tile([128, G], F32, tag="mu2")
            nc.vector.tensor_tensor(mu2[:], mu[:], mu[:], ALU.mult)
            ve = y_pool.tile([128, G], F32, tag="ve")
            nc.vector.tensor_scalar(ve[:], mu2[:], -1.0, LN_EPS, ALU.mult, ALU.add)
            nc.vector.tensor_tensor(ve[:], ve[:], ex2[:], ALU.add)
            # inv = rsqrt(ve): quake seed + 2 Newton iterations, all on DVE
            I32 = mybir.dt.int32
            ih = y_pool.tile([128, G], I32, tag="ih")
            nc.gpsimd.tensor_scalar(ih[:], ve[:].bitcast(I32), 1, None,
                                    ALU.arith_shift_right)
            x0 = y_pool.tile([128, G], I32, tag="x0")
            nc.gpsimd.tensor_scalar(x0[:], ih[:], -1, 0x5F3759DF, ALU.mult, ALU.add)
            xf = x0[:].bitcast(F32)
            cur = xf
            for it in range(2):
                aa = y_pool.tile([128, G], F32, tag=f"nw{it}a")
                nc.gpsimd.tensor_tensor(aa[:], cur, cur, ALU.mult)
                nc.gpsimd.tensor_tensor(aa[:], aa[:], ve[:], ALU.mult)
                nc.gpsimd.tensor_scalar(aa[:], aa[:], -0.5, 1.5, ALU.mult, ALU.add)
                xn = y_pool.tile([128, G], F32, tag=f"nw{it}x")
                nc.gpsimd.tensor_tensor(xn[:], cur, aa[:], ALU.mult)
                cur = xn[:]
            inv = cur
            y_stage = y_pool.tile([128, G, 128], F32, tag="ystage")
            for j in range(G):
                y1 = y_pool.tile([128, 128], F32, tag="y1", bufs=3)
                nc.gpsimd.scalar_tensor_tensor(
                    y1[:], tr_sb[:, j, :], mu[:, j:j + 1], wt_sb[:],
                    ALU.subtract, ALU.mult,
                )
                nc.gpsimd.scalar_tensor_tensor(
                    y_stage[:, j, :], y1[:], inv[:, j:j + 1], bt_sb[:],
                    ALU.mult, ALU.add,
                )
            for k in range(4):
                nc.gpsimd.dma_start(
                    y_d[gs:gs + G, :, 128 * k:128 * (k + 1)].rearrange("t c p -> c t p"),
                    y_stage[32 * k:32 * (k + 1), :, :],
                )

        h_hist = {}
        for _rep in range(reps):
            h_cur = h_pool.tile([128, 128], F32, tag="h", name=f"h0r{_rep}")
            nc.gpsimd.dma_start(h_cur[:], h0_d[:])
            gi_tiles.clear(); x_tiles.clear(); m_tiles.clear(); h_hist.clear()
            # prologue: chunk 0 inputs + gi, chunk 1 inputs
            load_chunk_inputs(0)
            load_chunk_inputs(1)
            for m in range(MT):
                phase1_block(0, m)

            for t in range(T):
                ci, tl = divmod(t, TC)
                if tl == 0:
                    load_chunk_inputs(ci + 2)
                phase2_step(t)
                for m in range(tl * MT // TC, (tl + 1) * MT // TC):
                    phase1_block(ci + 1, m)
                if t >= 15 and (t - 15) % 8 == 0:
                    phase3_group(t - 15)
            # drain remaining LN groups
            done = ((T - 16) // 8) * 8 + 8  # groups [0, done) emitted in-loop
            for gs in range(done, T, 8):
                phase3_group(gs)
            nc.gpsimd.dma_start(hT_d[:], h_cur[:])

    nc.compile()
    return nc


_CACHE = {}


def _get_program():
    if "nc" not in _CACHE:
        _CACHE["nc"] = build_program()
    return _CACHE["nc"]


def _fold_tb(a):
    """(rows, cols=BS) -> folded (128, 128): out[p, 32k+c] = a[128k+p, c]."""
    return a.reshape(4, 128, BS).transpose(1, 0, 2).reshape(128, 4 * BS)


def prep_in_maps(x, rnn_states, masks, W_ih, W_hh, b_ih, b_hh, ln_w, ln_b):
    x = np.asarray(x, np.float32)
    rnn_states = np.asarray(rnn_states, np.float32)
    masks = np.asarray(masks, np.float32)
    W_ih = np.asarray(W_ih, np.float32)
    W_hh = np.asarray(W_hh, np.float32)
    b_ih = np.asarray(b_ih, np.float32)
    b_hh = np.asarray(b_hh, np.float32)
    ln_w = np.asarray(ln_w, np.float32)
    ln_b = np.asarray(ln_b, np.float32)

    wihT = np.ascontiguousarray(W_ih.T).astype(BF)
    whhT = np.ascontiguousarray(W_hh.T).astype(BF)
    bias_g = np.concatenate([b_ih[:2 * H] + b_hh[:2 * H], b_ih[2 * H:]])
    bias_lhs = np.ascontiguousarray(bias_g[None, :]).astype(BF)
    bn_fold = np.ascontiguousarray(
        np.broadcast_to(b_hh[2 * H:].reshape(4, 128).T[:, :, None], (128, 4, BS))
        .reshape(128, 128), np.float32)
    smat = np.tile(np.eye(BS, dtype=np.float32), (4, 4))
    wt_fold = np.ascontiguousarray(
        np.broadcast_to(ln_w.reshape(4, 128)[:, None, :], (4, BS, 128))
        .reshape(128, 128), np.float32)
    bt_fold = np.ascontiguousarray(
        np.broadcast_to(ln_b.reshape(4, 128)[:, None, :], (4, BS, 128))
        .reshape(128, 128), np.float32)
    ident = np.eye(128, dtype=np.float32)

    in_maps = []
    for i in range(NCORES):
        sl = slice(BS * i, BS * (i + 1))
        xc = x[:, sl, :]  # (T, BS, I)
        x_fold = np.ascontiguousarray(
            xc.reshape(T, BS, 4, 128).transpose(0, 3, 2, 1).reshape(T, 128, 128)
        ).astype(BF)
        mc = masks[:, sl, 0]  # (T, BS)
        m_fold = np.ascontiguousarray(
            np.broadcast_to(mc[:, None, None, :], (T, 128, 4, BS)).reshape(T, 128, 128),
            np.float32)
        h0_fold = np.ascontiguousarray(_fold_tb(rnn_states[0, sl, :].T), np.float32)
        in_maps.append({
            "x_fold": x_fold, "m_fold": m_fold, "h0_fold": h0_fold,
            "wihT": wihT, "whhT": whhT, "bias_lhs": bias_lhs,
            "bn_fold": bn_fold, "smat": smat, "wt_fold": wt_fold,
            "bt_fold": bt_fold, "ident": ident,
        })

    return in_maps


def postprocess(results):
    y_full = np.empty((T, B, H), np.float32)
    hT_full = np.empty((1, B, H), np.float32)
    for i in range(NCORES):
        sl = slice(BS * i, BS * (i + 1))
        y_full[:, sl, :] = results[i]["y_out"]
        htf = results[i]["hT_out"]  # (128, 128) folded
        hT_full[0, sl, :] = htf.reshape(128, 4, BS).transpose(2, 1, 0).reshape(BS, H)
    return y_full, hT_full


def kernel(**inputs):
    nc = _get_program()
    in_maps = prep_in_maps(**inputs)
    res = run_bass_kernel_spmd(nc, in_maps, list(range(NCORES)))
    return postprocess(res.results)
